# revision 4
# baseline (speedup 1.0000x reference)
"""Additive (Bahdanau) attention on 8 Trainium2 NeuronCores.

Problem: B=64, S=2048, D=U=512
  Q = query @ W1                      (B, U)
  V = values @ W2                     (B, S, U)
  score = tanh(Q[:,None,:] + V) @ V1  (B, S, 1)
  attn = softmax(score, axis=1)
  out = sum(attn * values, axis=1)    (B, D)

Sharding: data-parallel over batch, 8 batches per core. Weights replicated.

Per-core kernel design (per batch):
  - values are fed host-transposed as valT (D, S) so the big matmul
    streams valT with W2 128x128 chunks stationary: V^T chunks land in
    PSUM as (128 u, 512 s) tiles -> contraction over D needs no on-chip
    transpose and DMA reads are 8KB-contiguous.
  - float32r (relaxed fp32) matmuls run the PE at 1 col/cycle (4x over
    plain fp32); measured rel err ~1e-4, far inside tolerance.
  - Q folds into the tanh as a per-partition ScalarE bias (u lives on
    partitions in the V^T orientation).
  - score = V1^T @ tanh_out via PE (V1 zero-padded to M=2: fp32r matmuls
    need even free dims). Scores land as (1, S) on partition 0.
  - softmax without max-subtraction (scores are tanh-bounded); Exp on
    ScalarE with fused accum_out gives the partial sums for free.
  - attn is broadcast to 128 partitions with a K=1 matmul whose
    stationary is (1/sum) replicated 128 wide, so normalization rides
    along: pb = rs_rep^T @ exp  ->  (128, 512) of normalized weights.
  - context^T[d] = sum_s valT[d,s] * attn[s] via VectorE
    scalar_tensor_tensor accum_out (the product itself goes to a
    stride-0 dummy so only the per-partition sums are materialized).
"""
import sys

sys.path.insert(0, "/opt/trn_rl_repo")

import numpy as np

B, S, D, U = 64, 2048, 512, 512
NCORES = 8
BL = B // NCORES
P = 128
KC = D // P     # 4 contraction chunks of the big matmul
UC = U // P     # 4 u chunks
NT = 512        # matmul free-dim tile (one fp32 PSUM bank)
ST = S // NT    # 4 s tiles

_compiled = None


def _build(repeat=1):
    import concourse.tile as tile
    from concourse import bacc, mybir, library_config

    F32R = mybir.dt.float32r
    F32 = mybir.dt.float32
    AF = mybir.ActivationFunctionType
    ALU = mybir.AluOpType

    nc = bacc.Bacc()
    vt_ext = nc.declare_dram_parameter("vt", [BL, D, S], F32R, isOutput=False)
    qt_ext = nc.declare_dram_parameter("qt", [D, BL], F32R, isOutput=False)
    w1_ext = nc.declare_dram_parameter("w1", [D, U], F32R, isOutput=False)
    w2_ext = nc.declare_dram_parameter("w2", [D, U], F32R, isOutput=False)
    v1_ext = nc.declare_dram_parameter("v1", [U, 2], F32R, isOutput=False)
    ones_ext = nc.declare_dram_parameter("ones", [1, P], F32R, isOutput=False)
    out_ext = nc.declare_dram_parameter("out", [BL, D], F32, isOutput=True)
    es_ext = nc.declare_dram_parameter("es", [BL, ST], F32, isOutput=True)

    with tile.TileContext(nc) as tc:
        with tc.tile_pool(name="consts", bufs=1) as consts, \
             tc.tile_pool(name="vtp", bufs=3) as vtp, \
             tc.tile_pool(name="thp", bufs=6) as thp, \
             tc.tile_pool(name="small", bufs=2) as small, \
             tc.tile_pool(name="pbp", bufs=3) as pbp, \
             tc.tile_pool(name="psv", bufs=5, space="PSUM") as psv, \
             tc.tile_pool(name="pssc", bufs=3, space="PSUM") as pssc:

            w1_sb = consts.tile([P, KC, U], F32R)
            nc.sync.dma_start(out=w1_sb[:], in_=w1_ext.rearrange("(kc ki) u -> ki kc u", ki=P))
            w2_sb = consts.tile([P, KC, U], F32R)
            nc.sync.dma_start(out=w2_sb[:], in_=w2_ext.rearrange("(kc ki) u -> ki kc u", ki=P))
            v1_sb = consts.tile([P, UC, 2], F32R)
            nc.sync.dma_start(out=v1_sb[:], in_=v1_ext.rearrange("(uc ui) o -> ui uc o", ui=P))
            qt_sb = consts.tile([P, KC, BL], F32R)
            nc.sync.dma_start(out=qt_sb[:], in_=qt_ext.rearrange("(kc ki) b -> ki kc b", ki=P))
            nc.gpsimd.load_library(library_config.attn)

            # QT[u, b] = (query @ W1)^T : per-partition bias source for tanh
            QT = consts.tile([P, UC, BL], F32)
            for uc in range(UC):
                pq = psv.tile([P, NT], F32, tag="v")
                for kc in range(KC):
                    nc.tensor.matmul(pq[:, :BL],
                                     w1_sb[:, kc, uc * P:(uc + 1) * P],
                                     qt_sb[:, kc, :],
                                     start=(kc == 0), stop=(kc == KC - 1))
                nc.vector.tensor_copy(out=QT[:, uc, :], in_=pq[:, :BL])

            def _batch_body(b):
                vt_sb = vtp.tile([P, KC, S], F32R, tag="vt")
                for kc in range(KC):
                    nc.sync.dma_start(out=vt_sb[:, kc, :],
                                      in_=vt_ext[b, kc * P:(kc + 1) * P, :])

                exp_flat = small.tile([1, S], F32R, tag="exp")
                es_parts = small.tile([1, ST], F32, tag="esum")
                ctx_parts = small.tile([P, KC * ST], F32, tag="ctxp")
                dummy = small.tile([P, 1], F32, tag="dummy")

                sc_tiles = {}

                def _tail(st):
                    sc = sc_tiles.pop(st)
                    nc.scalar.activation(out=exp_flat[0:1, st * NT:(st + 1) * NT],
                                         in_=sc[0:1, :], func=AF.Exp,
                                         accum_out=es_parts[0:1, st:st + 1])
                    pb = pbp.tile([P, NT], F32, tag="pb")
                    nc.gpsimd.partition_broadcast(
                        pb[:], exp_flat[0:1, st * NT:(st + 1) * NT].bitcast(F32))
                    for kc in range(KC):
                        nc.vector.scalar_tensor_tensor(
                            out=dummy[:].broadcast_to((P, NT)),
                            in0=vt_sb[:, kc, st * NT:(st + 1) * NT].bitcast(F32),
                            scalar=1.0, in1=pb[:],
                            op0=ALU.mult, op1=ALU.mult,
                            accum_out=ctx_parts[:, kc * ST + st:kc * ST + st + 1])

                def _emit_sc(st, uc, th):
                    if uc == 0:
                        sc = pssc.tile([2, NT], F32, tag="sc")
                        sc_tiles[st] = sc
                    nc.tensor.matmul(sc_tiles[st][:], v1_sb[:, uc, :], th[:],
                                     start=(uc == 0), stop=(uc == UC - 1))
                    if uc == UC - 1:
                        _tail(st)

                pending = []
                for st in range(ST):
                    for uc in range(UC):
                        pv = psv.tile([P, NT], F32, tag="v")
                        for kc in range(KC):
                            nc.tensor.matmul(pv[:],
                                             w2_sb[:, kc, uc * P:(uc + 1) * P],
                                             vt_sb[:, kc, st * NT:(st + 1) * NT],
                                             start=(kc == 0), stop=(kc == KC - 1))
                        th = thp.tile([P, NT], F32R, tag="th")
                        nc.scalar.activation(out=th[:], in_=pv[:], func=AF.Tanh,
                                             bias=QT[:, uc, b:b + 1], scale=1.0)
                        pending.append((st, uc, th))
                        if len(pending) > 1:
                            _emit_sc(*pending.pop(0))
                _emit_sc(*pending.pop(0))

                nc.sync.dma_start(out=es_ext[b, :], in_=es_parts[0:1, :])
                ctx_red = small.tile([P, KC], F32, tag="ctxr")
                nc.vector.tensor_reduce(
                    ctx_red[:],
                    ctx_parts[:].rearrange("p (kc st) -> p kc st", st=ST),
                    axis=mybir.AxisListType.X, op=ALU.add)
                nc.sync.dma_start(
                    out=out_ext[b, :].rearrange("(kc ki) -> ki kc", ki=P),
                    in_=ctx_red[:])

            if repeat == 1:
                for b in range(BL):
                    _batch_body(b)
            else:
                with tc.For_i(0, repeat, 1, staggered_reset=True):
                    for b in range(BL):
                        _batch_body(b)

    nc.compile()
    return nc


def _get_compiled():
    global _compiled
    if _compiled is None:
        _compiled = _build()
    return _compiled


OUTPUT_NAMES = ["out", "es"]


def make_in_maps(inputs):
    query = np.asarray(inputs["query"], dtype=np.float32)
    values = np.asarray(inputs["values"], dtype=np.float32)
    W1 = np.asarray(inputs["W1"], dtype=np.float32)
    W2 = np.asarray(inputs["W2"], dtype=np.float32)
    V1 = np.asarray(inputs["V1"], dtype=np.float32)

    v1pad = np.concatenate([V1, np.zeros((U, 1), np.float32)], axis=1)
    in_maps = []
    for c in range(NCORES):
        lo, hi = c * BL, (c + 1) * BL
        in_maps.append({
            "vt": np.ascontiguousarray(values[lo:hi].transpose(0, 2, 1)),
            "qt": np.ascontiguousarray(query[lo:hi].T),
            "w1": W1,
            "w2": W2,
            "v1": v1pad,
            "ones": np.ones((1, P), np.float32),
        })
    return in_maps


def postprocess_core0(outs):
    out = outs["out"]
    es = outs["es"]
    return out / es.sum(axis=1, keepdims=True)


def kernel(query, values, W1, W2, V1, trace=False):
    from concourse.bass_utils import run_bass_kernel_spmd

    nc = _get_compiled()
    in_maps = make_in_maps({"query": query, "values": values,
                            "W1": W1, "W2": W2, "V1": V1})

    res = run_bass_kernel_spmd(nc, in_maps, list(range(NCORES)), trace=trace)
    out = np.concatenate([res.results[c]["out"] for c in range(NCORES)], axis=0)
    es = np.concatenate([res.results[c]["es"] for c in range(NCORES)], axis=0)
    out = out / es.sum(axis=1, keepdims=True)
    if trace:
        return out, res
    return out

